# revision 11
# baseline (speedup 1.0000x reference)
"""Trainium2 Bass kernel for a DoReFa-quantized ResNet BasicBlock.

  out = act( bn2(conv3x3(act(bn1(conv3x3(x, q(w1)))), q(w2))) + x )

with 4-bit DoReFa weight quantization q(.) and 4-bit activation
quantization act(.) (clip to [0,1], round to k/15 grid), BN with fixed
running stats.

Key facts exploited:
  * q(w) = w_int/15 with w_int odd integers in [-15, 15]  -> exact in bf16
  * act(.) output = a_int/15 with a_int in {0..15}        -> exact in bf16
  * products (<=225) and partial sums (<=129600 < 2^24) are exactly
    representable in fp32, so conv2 in bf16 is EXACT integer arithmetic.
  * conv1's continuous input x is split x = x_hi + x_lo (both bf16);
    two bf16 matmuls give ~2^-17 relative accuracy.
  * BN folds into per-output-channel scale/bias applied to the PSUM
    result; clip via min/max; round-half-even via the +-1.5*2^23 trick.

Distribution: data-parallel over batch, 4 images per core (8 cores).
On-chip layout: 2 images stacked on the 128 partitions (64 channels
each); conv = 9 matmuls (one per 3x3 tap) with block-diagonal [128,128]
weights, spatial pixels in the free dimension.  Images live in SBUF as
zero-padded 114x114 rows so every tap is just a free-dim offset; one
PSUM bank holds 4 output rows (456 elements incl. left/right pads).
"""

import sys
import os
import numpy as np

for _p in ("/opt/trn_rl_repo", "/root/.axon_site/_ro/trn_rl_repo"):
    if os.path.isdir(_p) and _p not in sys.path:
        sys.path.insert(0, _p)

import ml_dtypes  # noqa: E402

# ----------------------------------------------------------------------------
# problem constants (hardcoded per spec)
C = 64
H = W = 112
N_FULL = 32
NCORES = 8
N_PER_CORE = N_FULL // NCORES        # 4 images
NPAIRS = N_PER_CORE // 2             # 2 image pairs (2 images share 128 parts)
WP = 114                             # padded row width
LEAD = 1                             # one leading zero element
TLEN = LEAD + WP * WP + 2            # padded image + tail slack = 12999
HWPIX = H * W                        # 12544
ROWS_PER_CHUNK = 4
NCHUNKS = H // ROWS_PER_CHUNK        # 28
CHUNK_N = ROWS_PER_CHUNK * W         # 448 psum elements (pad cols excluded)
PIECE_ROWS = [8, 8, 16, 28, 28, 24]  # x load/convert piece sizes (rows)
PIECE_START = [0, 8, 16, 32, 60, 88]
# piece p is prefetched while chunk PIECE_EMIT[p] runs
PIECE_EMIT = {1: 3, 7: 4, 14: 5}
CHUNK_N2 = ROWS_PER_CHUNK * WP       # 456: conv2 psum incl pad cols (dr mode)
MAGIC = 12582912.0                   # 1.5 * 2**23, forces RNE round-to-int
EPS = 1e-5

_CACHE = {}


# ----------------------------------------------------------------------------
# host-side preprocessing
def _quantize_weight_int(w):
    """DoReFa 4-bit weight quantization, returning 15*q(w) (odd ints in
    [-15,15]) as float32.  Computed with jax on CPU to match the
    reference bit-for-bit."""
    import jax
    import jax.numpy as jnp

    cpu = jax.devices("cpu")[0]
    with jax.default_device(cpu):
        t = jnp.tanh(jnp.asarray(np.asarray(w, dtype=np.float32)))
        t = t / (2.0 * jnp.max(jnp.abs(t))) + 0.5
        m = jnp.round(t * 15.0)
        out = 2.0 * m - 15.0
        return np.asarray(out, dtype=np.float32)


def _block_diag_taps(w_int):
    """w_int: (Cout, Cin, 3, 3) float int values.  Returns (128, 9*128)
    bf16: for tap t, lhsT[k, t*128+m] with block-diagonal two-image
    structure; lhsT[cin, cout] = w[cout, cin] in each 64x64 block."""
    bd = np.zeros((128, 9, 128), dtype=np.float32)
    for t in range(9):
        ky, kx = divmod(t, 3)
        blk = w_int[:, :, ky, kx].T  # (Cin, Cout)
        bd[0:64, t, 0:64] = blk
        bd[64:128, t, 64:128] = blk
    return bd.reshape(128, 9 * 128).astype(ml_dtypes.bfloat16)


def _dr_weights(w_int):
    """fp8 DoubleRow conv2 weights: per kx, taps (ky=0, ky=1) interleaved
    [k, 2, m]; plus plain [k, m] blocks for ky=2."""
    f8 = ml_dtypes.float8_e4m3fn
    dr = np.zeros((128, 3, 2, 128), dtype=np.float32)
    ky2 = np.zeros((128, 3, 128), dtype=np.float32)
    for kx in range(3):
        for i in range(2):
            blk = w_int[:, :, i, kx].T
            dr[0:64, kx, i, 0:64] = blk
            dr[64:128, kx, i, 64:128] = blk
        blk2 = w_int[:, :, 2, kx].T
        ky2[0:64, kx, 0:64] = blk2
        ky2[64:128, kx, 64:128] = blk2
    return dr.reshape(128, 768).astype(f8), ky2.reshape(128, 384).astype(f8)


def _drc_weights(w_int):
    """conv2 column-pair DR weights: lanes = taps (ky=2, kx=0) and
    (ky=2, kx=1); the ifmap lanes are the same a1 row shifted one col."""
    f8 = ml_dtypes.float8_e4m3fn
    c = np.zeros((128, 2, 128), dtype=np.float32)
    for i in range(2):
        blk = w_int[:, :, 2, i].T
        c[0:64, i, 0:64] = blk
        c[64:128, i, 64:128] = blk
    return c.reshape(128, 256).astype(f8)


def _dr1_weights(w_int):
    """conv1 DR weights: per tap, both lanes carry the same block-diag
    [k, 2, m] (lanes multiply the h1/h2 fp8 parts of x)."""
    f8 = ml_dtypes.float8_e4m3fn
    dr = np.zeros((128, 9, 2, 128), dtype=np.float32)
    for t in range(9):
        ky, kx = divmod(t, 3)
        blk = w_int[:, :, ky, kx].T
        for i in range(2):
            dr[0:64, t, i, 0:64] = blk
            dr[64:128, t, i, 64:128] = blk
    return dr.reshape(128, 9 * 256).astype(f8)


def _bn_fold(gamma, beta, mean, var):
    gamma = np.asarray(gamma, np.float32)
    beta = np.asarray(beta, np.float32)
    mean = np.asarray(mean, np.float32)
    var = np.asarray(var, np.float32)
    inv = gamma / np.sqrt(var + np.float32(EPS))
    b = beta - mean * inv
    return inv.astype(np.float32), b.astype(np.float32)


def _tile2(v):
    """(64,) -> (128,1) stacked for the two images on the partitions."""
    return np.concatenate([v, v]).reshape(128, 1).astype(np.float32)


# ----------------------------------------------------------------------------
# walrus workaround: CTRL instructions (Drain) accept only ONE sync wait
def _install_tile_patch(tile_mod):
    import bass_rust

    if getattr(tile_mod.TileContext, "_drain_split_patch", False):
        return

    def _patched(self, tick_clock, wait_clock):
        nc = self.nc
        drain_inst = nc.sync.drain()
        wait_clock.add_sem_waits(
            drain_inst.ins, bass_rust.ScopedClock({None: tick_clock.global_clock})
        )
        si = drain_inst.ins.sync_info
        waits = list(si.on_wait or []) if si is not None else []
        if len(waits) > 1:
            si.on_wait = waits[:1]
            for w in waits[1:]:
                d2 = nc.sync.drain()
                d2.ins.sync_info = bass_rust.SyncInfo(on_wait=[w], on_update=[])
        nc.all_engine_barrier()
        assert self.sems is not None
        popped = nc._tile_sem_poison_stack.pop()
        assert popped is self._sem_poison
        nc.clear_and_free_semaphores(list(self.sems.allocated().values()))
        nc.all_engine_barrier()

    tile_mod.TileContext._drain_and_barrier = _patched
    tile_mod.TileContext._drain_split_patch = True


def _split_multiwaits(nc, max_waits=1):
    """This walrus build allows only ONE sync wait per instruction
    (setupSyncWait raises 'Too many sync wait commands' at 2 for both
    CTRL and MM structs).  Move excess waits onto injected same-engine
    NoOps immediately before the over-limit instruction — semantically
    identical since the engine executes its stream in order."""
    import bass_rust
    from concourse import mybir

    n_split = 0
    for fn in nc.m.functions:
        for bb in fn.blocks:
            insts = bb.instructions  # live list
            new = None
            for idx, inst in enumerate(insts):
                si = inst.sync_info
                waits = list(si.on_wait) if (si is not None and si.on_wait) else []
                if len(waits) <= max_waits:
                    if new is not None:
                        new.append(inst)
                    continue
                if new is None:
                    new = list(insts[:idx])
                excess, keep = waits[:-max_waits], waits[-max_waits:]
                for w in excess:
                    nop = mybir.InstNoOp(
                        name=nc.get_next_instruction_name(), ins=[], outs=[]
                    )
                    nop.engine = inst.engine
                    nop.sync_info = bass_rust.SyncInfo(on_wait=[w], on_update=[])
                    nc.register_instruction(nop, overwrite=True)
                    new.append(nop)
                si.on_wait = keep
                new.append(inst)
                n_split += 1
            if new is not None:
                bb.instructions = new
    return n_split


# ----------------------------------------------------------------------------
# device program
DEFAULT_MODE = os.environ.get("BASS_CONV1_MODE", "f16")


def build_program(repeat=1, mode=None):
    mode = mode or DEFAULT_MODE
    key = ("nc", repeat, mode)
    if key in _CACHE:
        return _CACHE[key]

    import concourse.bass as bass
    import concourse.tile as tile
    from concourse import mybir

    _install_tile_patch(tile)

    f32 = mybir.dt.float32
    bf16 = mybir.dt.bfloat16
    Alu = mybir.AluOpType
    Act = mybir.ActivationFunctionType

    nc = bass.Bass("TRN2", debug=False, enable_asserts=False)

    f32r = mybir.dt.float32r
    fp8 = mybir.dt.float8e4
    f16 = mybir.dt.float16
    dr = (mode in ("split_dr", "dr_mid", "f16"))
    drmid = (mode == "dr_mid")
    w1dt = f32r if mode == "fp32r" else bf16
    xdt = {"fp32r": f32r, "f16": f16}.get(mode, f32)
    xd = nc.dram_tensor("x", [NPAIRS, 128, HWPIX], xdt, kind="ExternalInput").ap()
    if drmid:
        w1drd = nc.dram_tensor("w1dr", [128, 9 * 256], fp8,
                               kind="ExternalInput").ap()
    w1d = nc.dram_tensor("w1bd", [128, 9 * 128], w1dt, kind="ExternalInput").ap()
    if dr:
        w2d = nc.dram_tensor("w2dr", [128, 768], fp8, kind="ExternalInput").ap()
        w2k2d = nc.dram_tensor("w2ky2", [128, 384], fp8, kind="ExternalInput").ap()
        w2cd = nc.dram_tensor("w2c", [128, 256], fp8, kind="ExternalInput").ap()
    else:
        w2d = nc.dram_tensor("w2bd", [128, 9 * 128], bf16, kind="ExternalInput").ap()
    s1d = nc.dram_tensor("s1", [128, 1], f32, kind="ExternalInput").ap()
    b1d = nc.dram_tensor("b1c", [128, 1], f32, kind="ExternalInput").ap()
    s2d = nc.dram_tensor("s2", [128, 1], f32, kind="ExternalInput").ap()
    b2d = nc.dram_tensor("b2c", [128, 1], f32, kind="ExternalInput").ap()
    odt = fp8 if mode == "f16" else f32  # f16 mode: 15*out ints, host /15
    od = nc.dram_tensor("out", [NPAIRS, 128, HWPIX], odt, kind="ExternalOutput").ap()

    def img_rows(tile_ap, r0, r1, c0=1, c1=113):
        """strided view of padded-image tile rows [r0,r1) cols [c0,c1)"""
        flat = tile_ap[:, LEAD + r0 * WP: LEAD + r1 * WP]
        return flat.rearrange("p (r c) -> p r c", c=WP)[:, :, c0:c1]

    def img_rows_at(tile_ap, base, r0, r1, c0=1, c1=113):
        flat = tile_ap[:, base + LEAD + r0 * WP: base + LEAD + r1 * WP]
        return flat.rearrange("p (r c) -> p r c", c=WP)[:, :, c0:c1]

    def tap_rhs(tile_ap, y0, ky, kx):
        """[128, 4, 112] matmul rhs for tap (ky,kx) of out rows y0..y0+3"""
        off = LEAD + (y0 + ky) * WP + kx
        flat = tile_ap[:, off: off + ROWS_PER_CHUNK * WP]
        return flat.rearrange("p (r c) -> p r c", c=WP)[:, :, 0:112]

    with tile.TileContext(nc) as tc:
        import contextlib

        with contextlib.ExitStack() as ctx:
            consts = ctx.enter_context(tc.tile_pool(name="consts", bufs=1))
            bigp = ctx.enter_context(tc.tile_pool(name="big", bufs=1))
            xpp = ctx.enter_context(tc.tile_pool(name="xp", bufs=2))
            workp = ctx.enter_context(tc.tile_pool(name="work", bufs=2))
            outp = ctx.enter_context(tc.tile_pool(name="outs", bufs=4))
            ps1p = ctx.enter_context(tc.tile_pool(name="ps1", bufs=4, space="PSUM"))
            ps2p = ctx.enter_context(tc.tile_pool(name="ps2", bufs=4, space="PSUM"))

            w1t = consts.tile([128, 9 * 128], w1dt, tag="w1t")
            nc.sync.dma_start(w1t[:], w1d[:])
            if drmid:
                w1drt = consts.tile([128, 9 * 256], fp8, tag="w1drt")
                nc.sync.dma_start(w1drt[:], w1drd[:])
            if dr:
                w2t = consts.tile([128, 768], fp8, tag="w2t")
                nc.sync.dma_start(w2t[:], w2d[:])
                w2k2t = consts.tile([128, 384], fp8, tag="w2k2t")
                nc.sync.dma_start(w2k2t[:], w2k2d[:])
                w2ct = consts.tile([128, 256], fp8, tag="w2ct")
                nc.sync.dma_start(w2ct[:], w2cd[:])
            else:
                w2t = consts.tile([128, 9 * 128], bf16, tag="w2t")
                nc.sync.dma_start(w2t[:], w2d[:])
            s1t = consts.tile([128, 1], f32, tag="s1t")
            nc.sync.dma_start(s1t[:], s1d[:])
            b1t = consts.tile([128, 1], f32, tag="b1t")
            nc.sync.dma_start(b1t[:], b1d[:])
            s2t = consts.tile([128, 1], f32, tag="s2t")
            nc.sync.dma_start(s2t[:], s2d[:])
            b2t = consts.tile([128, 1], f32, tag="b2t")
            nc.sync.dma_start(b2t[:], b2d[:])

            for pair_rep in range(NPAIRS * repeat):
                pair = pair_rep % NPAIRS
                if drmid:
                    h12 = bigp.tile([128, 2 * TLEN], fp8, tag="h12")
                    lo2 = bigp.tile([128, TLEN], bf16, tag="lo2")
                    pad_tiles = (lo2,)
                elif mode == "f16":
                    xf = bigp.tile([128, TLEN], f16, tag="xf", bufs=2)
                    pad_tiles = (xf,)
                elif mode != "fp32r":
                    # double-buffered so pair p+1's x conversion overlaps
                    # pair p's tail instead of WAR-stalling behind it
                    xhi = bigp.tile([128, TLEN], bf16, tag="xhi", bufs=2)
                    xlo = bigp.tile([128, TLEN], bf16, tag="xlo", bufs=2)
                    pad_tiles = (xhi, xlo)
                else:
                    xt = bigp.tile([128, TLEN], f32r, tag="xt")
                    pad_tiles = (xt,)
                if dr:
                    a1 = bigp.tile([128, TLEN], fp8, tag="a1")
                else:
                    a1 = bigp.tile([128, TLEN], bf16, tag="a1")

                # zero pad regions (lead + row0, row113 + tail, col 0, col 113)
                def _ms(ap):
                    if ap.dtype == f32r:
                        ap = ap.bitcast(f32)
                    nc.gpsimd.memset(ap, 0.0)

                def _zero_pads(t_, base):
                    _ms(t_[:, base: base + LEAD + WP])
                    _ms(t_[:, base + LEAD + 113 * WP: base + TLEN])
                    colv = t_[:, base + LEAD: base + LEAD + WP * WP].rearrange(
                        "p (r c) -> p r c", c=WP
                    )
                    _ms(colv[:, :, 0:1])
                    _ms(colv[:, :, 113:114])

                for t_ in pad_tiles:
                    _zero_pads(t_, 0)
                if drmid:
                    _zero_pads(h12, 0)
                    _zero_pads(h12, TLEN)
                _zero_pads(a1, 0)

                def convert_piece(p):
                    pr, p0 = PIECE_ROWS[p], PIECE_START[p]
                    src_flat = xd[pair][:, p0 * W: (p0 + pr) * W]
                    src_ap = src_flat.rearrange("p (r c) -> p r c", c=W)
                    r0 = p0 + 1  # tile row of first image row
                    if mode == "f16":
                        nc.sync.dma_start(img_rows(xf, r0, r0 + pr), src_ap)
                    elif drmid:
                        xp_ = xpp.tile([128, 28 * W], f32, tag="xp")
                        nc.sync.dma_start(xp_[:, 0: pr * W], src_flat)
                        xpv = xp_[:, 0: pr * W].rearrange("p (r c) -> p r c", c=W)
                        h1v = img_rows(h12, r0, r0 + pr)
                        nc.gpsimd.dma_start(h1v, src_ap)  # cast f32->fp8
                        ra = xpp.tile([128, 28 * W], f32, tag="ra")
                        rav = ra[:, 0: pr * W].rearrange("p (r c) -> p r c", c=W)
                        nc.gpsimd.tensor_sub(rav, xpv, h1v)
                        h2v = img_rows_at(h12, TLEN, r0, r0 + pr)
                        nc.scalar.activation(h2v, rav, Act.Copy, bias=0.0, scale=1.0)
                        lo2v = img_rows(lo2, r0, r0 + pr)
                        nc.vector.tensor_sub(lo2v, rav, h2v)
                    elif mode != "fp32r":
                        xp_ = xpp.tile([128, 28 * W], f32, tag="xp")
                        nc.sync.dma_start(xp_[:, 0: pr * W], src_flat)
                        xpv = xp_[:, 0: pr * W].rearrange("p (r c) -> p r c", c=W)
                        hiv = img_rows(xhi, r0, r0 + pr)
                        lov = img_rows(xlo, r0, r0 + pr)
                        # casting DMA (fp32->bf16) produces x_hi; x_lo picks
                        # up whatever rounding the cast used.
                        nc.gpsimd.dma_start(hiv, src_ap)
                        nc.vector.tensor_sub(lov, xpv, hiv)
                    else:
                        nc.sync.dma_start(img_rows(xt, r0, r0 + pr), src_ap)

                def conv1_chunk(k):
                    y0 = k * ROWS_PER_CHUNK
                    CN1 = CHUNK_N2 if drmid else CHUNK_N
                    ps = ps1p.tile([128, CN1], f32, tag="ps1")
                    if drmid:
                        h12two = h12[:].rearrange("p (two L) -> p two L", two=2)
                        for t in range(9):
                            ky, kx = divmod(t, 3)
                            off = (y0 + ky) * WP + kx
                            nc.tensor.matmul(
                                ps[:],
                                w1drt[:, t * 256: (t + 1) * 256].rearrange(
                                    "p (two m) -> p two m", two=2),
                                h12two[:, :, off: off + CHUNK_N2],
                                start=(t == 0),
                                stop=False,
                                perf_mode=mybir.MatmulPerfMode.DoubleRow,
                            )
                        for t in range(9):
                            ky, kx = divmod(t, 3)
                            off = (y0 + ky) * WP + kx
                            nc.tensor.matmul(
                                ps[:],
                                w1t[:, t * 128: (t + 1) * 128],
                                lo2[:, off: off + CHUNK_N2],
                                start=False,
                                stop=(t == 8),
                            )
                    else:
                        srcs = ((xf,) if mode == "f16" else
                                (xt,) if mode == "fp32r" else (xhi, xlo))
                        n_mm = 9 * len(srcs)
                        i = 0
                        for t in range(9):
                            ky, kx = divmod(t, 3)
                            for s_ in srcs:
                                nc.tensor.matmul(
                                    ps[:],
                                    w1t[:, t * 128: (t + 1) * 128],
                                    tap_rhs(s_, y0, ky, kx),
                                    start=(i == 0),
                                    stop=(i == n_mm - 1),
                                )
                                i += 1
                    # t1 = psum * inv1 + 15*b1; then +MAGIC rounds to
                    # nearest-even integer, clip in the shifted domain,
                    # subtract MAGIC (round/clip commute here).
                    t1 = workp.tile([128, CN1], f32, tag="t1")
                    nc.scalar.activation(
                        t1[:], ps[:], Act.Identity, bias=b1t[:], scale=s1t[:]
                    )
                    m1 = workp.tile([128, CN1], f32, tag="m1")
                    nc.vector.tensor_scalar(
                        m1[:], t1[:], MAGIC, MAGIC, Alu.add, Alu.max
                    )
                    a1v = img_rows(a1, y0 + 1, y0 + 5)
                    if drmid:
                        m1v = m1[:].rearrange("p (r c) -> p r c", c=WP)[:, :, 1:113]
                    else:
                        m1v = m1[:].rearrange("p (r c) -> p r c", c=W)
                    nc.vector.tensor_scalar(
                        a1v, m1v, MAGIC + 15.0, -MAGIC, Alu.min, Alu.add
                    )


                def conv2_chunk(k):
                    y0 = k * ROWS_PER_CHUNK
                    if dr:
                        import bass_rust as _br
                        ps = ps2p.tile([128, CHUNK_N2], f32, tag="ps2")
                        for kx in range(3):
                            off = y0 * WP + kx
                            rrhs = a1[:, off: off + WP + CHUNK_N2].copy()
                            rrhs.ap = _br.VecI64Pair(
                                [[TLEN, 128], [WP, 2], [1, CHUNK_N2]])
                            nc.tensor.matmul(
                                ps[:],
                                w2t[:, kx * 256: (kx + 1) * 256].rearrange(
                                    "p (two m) -> p two m", two=2),
                                rrhs,
                                start=(kx == 0),
                                stop=False,
                                perf_mode=mybir.MatmulPerfMode.DoubleRow,
                            )
                        # (ky=2, kx=0|1) as one DR matmul with overlapping
                        # lanes: lane i reads a1 at a one-element offset
                        off = (y0 + 2) * WP
                        crhs = a1[:, off: off + CHUNK_N2 + 1].copy()
                        crhs.ap = _br.VecI64Pair(
                            [[TLEN, 128], [1, 2], [1, CHUNK_N2]])
                        nc.tensor.matmul(
                            ps[:],
                            w2ct[:].rearrange("p (two m) -> p two m", two=2),
                            crhs,
                            start=False, stop=False,
                            perf_mode=mybir.MatmulPerfMode.DoubleRow,
                        )
                        off2 = (y0 + 2) * WP + 2
                        nc.tensor.matmul(
                            ps[:],
                            w2k2t[:, 2 * 128: 3 * 128],
                            a1[:, off2: off2 + CHUNK_N2],
                            start=False,
                            stop=True,
                        )
                    else:
                        ps = ps2p.tile([128, CHUNK_N], f32, tag="ps2")
                        for t in range(9):
                            ky, kx = divmod(t, 3)
                            nc.tensor.matmul(
                                ps[:],
                                w2t[:, t * 128: (t + 1) * 128],
                                tap_rhs(a1, y0, ky, kx),
                                start=(t == 0),
                                stop=(t == 8),
                            )
                    NC2 = CHUNK_N2 if dr else CHUNK_N
                    # t2 = psum * (inv2/15) + 15*b2
                    t2 = workp.tile([128, NC2], f32, tag="t2")
                    nc.scalar.activation(
                        t2[:], ps[:], Act.Identity, bias=b2t[:], scale=s2t[:]
                    )
                    # + 15*x  (x = x_hi + x_lo)
                    if drmid:
                        xbase = (y0 + 1) * WP + 1  # center tap, incl pad cols
                        v1 = workp.tile([128, NC2], f32, tag="v1")
                        nc.vector.scalar_tensor_tensor(
                            v1[:], h12[:, xbase: xbase + CHUNK_N2], 15.0, t2[:],
                            Alu.mult, Alu.add)
                        v0 = workp.tile([128, NC2], f32, tag="v0")
                        nc.vector.scalar_tensor_tensor(
                            v0[:], h12[:, TLEN + xbase: TLEN + xbase + CHUNK_N2],
                            15.0, v1[:], Alu.mult, Alu.add)
                        v2 = workp.tile([128, NC2], f32, tag="v2")
                        nc.vector.scalar_tensor_tensor(
                            v2[:], lo2[:, xbase: xbase + CHUNK_N2], 15.0, v0[:],
                            Alu.mult, Alu.add)
                    elif mode == "f16":
                        xbase = (y0 + 1) * WP + 1  # center tap, incl pad cols
                        v2 = workp.tile([128, NC2], f32, tag="v2")
                        nc.vector.scalar_tensor_tensor(
                            v2[:], xf[:, xbase: xbase + CHUNK_N2], 15.0, t2[:],
                            Alu.mult, Alu.add)
                    elif dr:
                        xbase = (y0 + 1) * WP + 1  # center tap, incl pad cols
                        xh = xhi[:, xbase: xbase + CHUNK_N2]
                        xl = xlo[:, xbase: xbase + CHUNK_N2]
                        v1 = workp.tile([128, NC2], f32, tag="v1")
                        nc.vector.scalar_tensor_tensor(
                            v1[:], xh, 15.0, t2[:], Alu.mult, Alu.add)
                        v2 = workp.tile([128, NC2], f32, tag="v2")
                        nc.vector.scalar_tensor_tensor(
                            v2[:], xl, 15.0, v1[:], Alu.mult, Alu.add)
                    elif mode == "split":
                        t2v = t2[:].rearrange("p (r c) -> p r c", c=W)
                        v1 = workp.tile([128, NC2], f32, tag="v1")
                        nc.vector.scalar_tensor_tensor(
                            v1[:].rearrange("p (r c) -> p r c", c=W),
                            tap_rhs(xhi, y0, 1, 1), 15.0, t2v,
                            Alu.mult, Alu.add,
                        )
                        v2 = workp.tile([128, NC2], f32, tag="v2")
                        nc.vector.scalar_tensor_tensor(
                            v2[:].rearrange("p (r c) -> p r c", c=W),
                            tap_rhs(xlo, y0, 1, 1), 15.0,
                            v1[:].rearrange("p (r c) -> p r c", c=W),
                            Alu.mult, Alu.add,
                        )
                    else:
                        t2v = t2[:].rearrange("p (r c) -> p r c", c=W)
                        v2 = workp.tile([128, NC2], f32, tag="v2")
                        nc.vector.scalar_tensor_tensor(
                            v2[:].rearrange("p (r c) -> p r c", c=W),
                            tap_rhs(xt, y0, 1, 1).bitcast(f32), 15.0, t2v,
                            Alu.mult, Alu.add,
                        )
                    u2 = workp.tile([128, NC2], f32, tag="u2")
                    if mode == "f16":
                        # clip on the (otherwise idle) Pool engine; round via
                        # +MAGIC-MAGIC fused in one 2-ALU DVE op, fp8 output
                        # of the integer 15*out (host divides by 15).
                        nc.gpsimd.tensor_scalar(
                            u2[:], v2[:], 0.0, 15.0, Alu.max, Alu.min)
                        o = outp.tile([128, NC2], fp8, tag="o")
                        nc.vector.tensor_scalar(
                            o[:], u2[:], MAGIC, -MAGIC, Alu.add, Alu.add)
                    else:
                        nc.vector.tensor_scalar(
                            u2[:], v2[:], 0.0, 15.0, Alu.max, Alu.min)
                        m2 = workp.tile([128, NC2], f32, tag="m2")
                        nc.scalar.activation(
                            m2[:], u2[:], Act.Copy, bias=MAGIC, scale=1.0)
                        o = outp.tile([128, NC2], f32, tag="o")
                        nc.vector.tensor_scalar(
                            o[:], m2[:], -MAGIC, 1.0 / 15.0, Alu.add, Alu.mult
                        )
                    if dr:
                        ov = o[:].rearrange("p (r c) -> p r c", c=WP)[:, :, 1:113]
                        dst = od[pair].rearrange(
                            "p (r c) -> p r c", c=W)[:, y0: y0 + 4, :]
                        nc.sync.dma_start(dst, ov)
                    else:
                        nc.sync.dma_start(
                            od[pair][:, y0 * W: y0 * W + CHUNK_N], o[:]
                        )

                convert_piece(0)
                convert_piece(1)
                convert_piece(2)
                conv1_chunk(0)
                for k in range(1, NCHUNKS):
                    if k in PIECE_EMIT:
                        convert_piece(PIECE_EMIT[k])
                    conv1_chunk(k)
                    conv2_chunk(k - 1)
                conv2_chunk(NCHUNKS - 1)

    _split_multiwaits(nc)
    _CACHE[key] = nc
    return nc


# ----------------------------------------------------------------------------
def _make_in_maps(x, w1, gamma1, beta1, mean1, var1, w2, gamma2, beta2,
                  mean2, var2, mode=None):
    mode = mode or DEFAULT_MODE
    x = np.ascontiguousarray(np.asarray(x, dtype=np.float32))
    w1i = _quantize_weight_int(np.asarray(w1))
    w2i = _quantize_weight_int(np.asarray(w2))
    w1bd = _block_diag_taps(w1i)
    if mode == "fp32r":
        w1bd = np.asarray(w1bd, dtype=np.float32)
    w2bd = _block_diag_taps(w2i)
    inv1, b1 = _bn_fold(gamma1, beta1, mean1, var1)
    inv2, b2 = _bn_fold(gamma2, beta2, mean2, var2)
    s1 = _tile2(inv1)                      # psum1 * inv1 + 15*b1
    b1c = _tile2(np.float32(15.0) * b1)
    s2 = _tile2(inv2 / np.float32(15.0))   # psum2 * inv2/15 + 15*b2
    b2c = _tile2(np.float32(15.0) * b2)

    if mode in ("split_dr", "dr_mid", "f16"):
        w2dr, w2ky2 = _dr_weights(w2i)
    if mode == "f16":
        x = x.astype(np.float16)
    in_maps = []
    for core in range(NCORES):
        xs = x[core * N_PER_CORE: (core + 1) * N_PER_CORE]
        xs = xs.reshape(NPAIRS, 128, HWPIX)
        m = {"x": xs, "w1bd": w1bd,
             "s1": s1, "b1c": b1c, "s2": s2, "b2c": b2c}
        if mode in ("split_dr", "dr_mid", "f16"):
            m["w2dr"] = w2dr
            m["w2ky2"] = w2ky2
            m["w2c"] = _drc_weights(w2i)
        else:
            m["w2bd"] = w2bd
        if mode == "dr_mid":
            m["w1dr"] = _dr1_weights(w1i)
        in_maps.append(m)
    return in_maps


def _install_hook_logging():
    """Surface neuronx-cc hook exceptions (PJRT swallows the traceback)."""
    if _CACHE.get("hooked"):
        return
    import traceback
    from concourse import bass2jax

    bass2jax.install_neuronx_cc_hook()
    try:
        import libneuronxla
    except ImportError:
        return
    orig = libneuronxla.neuronx_cc

    def wrapped(*a, **k):
        try:
            return orig(*a, **k)
        except BaseException:
            traceback.print_exc()
            raise

    libneuronxla.neuronx_cc = wrapped
    bass2jax.install_neuronx_cc_hook = lambda: None
    _CACHE["hooked"] = True


def run(in_maps, trace=False, mode=None):
    from concourse import bass_utils

    _install_hook_logging()
    nc = build_program(mode=mode)
    res = bass_utils.run_bass_kernel_spmd(
        nc, in_maps, core_ids=list(range(NCORES)), trace=trace
    )
    return res


def unshard(res, mode=None):
    mode = mode or DEFAULT_MODE
    outs = []
    for core in range(NCORES):
        o = np.asarray(res.results[core]["out"], dtype=np.float32)
        if mode == "f16":  # device wrote integer 15*out as fp8
            o *= np.float32(1.0 / 15.0)
        outs.append(o.reshape(N_PER_CORE, C, H, W))
    return np.concatenate(outs, axis=0)


def kernel(x, w1, gamma1, beta1, mean1, var1, w2, gamma2, beta2, mean2, var2):
    in_maps = _make_in_maps(x, w1, gamma1, beta1, mean1, var1,
                            w2, gamma2, beta2, mean2, var2)
    res = run(in_maps, trace=False)  # DEFAULT_MODE for both maps and program
    return unshard(res)



# revision 12
# speedup vs baseline: 9.7524x; 9.7524x over previous
"""Trainium2 Bass kernel for a DoReFa-quantized ResNet BasicBlock.

  out = act( bn2(conv3x3(act(bn1(conv3x3(x, q(w1)))), q(w2))) + x )

with 4-bit DoReFa weight quantization q(.) and 4-bit activation
quantization act(.) (clip to [0,1], round to k/15 grid), BN with fixed
running stats.

Key facts exploited:
  * q(w) = w_int/15 with w_int odd integers in [-15, 15]  -> exact in bf16
  * act(.) output = a_int/15 with a_int in {0..15}        -> exact in bf16
  * products (<=225) and partial sums (<=129600 < 2^24) are exactly
    representable in fp32, so conv2 in bf16 is EXACT integer arithmetic.
  * conv1's continuous input x is split x = x_hi + x_lo (both bf16);
    two bf16 matmuls give ~2^-17 relative accuracy.
  * BN folds into per-output-channel scale/bias applied to the PSUM
    result; clip via min/max; round-half-even via the +-1.5*2^23 trick.

Distribution: data-parallel over batch, 4 images per core (8 cores).
On-chip layout: 2 images stacked on the 128 partitions (64 channels
each); conv = 9 matmuls (one per 3x3 tap) with block-diagonal [128,128]
weights, spatial pixels in the free dimension.  Images live in SBUF as
zero-padded 114x114 rows so every tap is just a free-dim offset; one
PSUM bank holds 4 output rows (456 elements incl. left/right pads).
"""

import sys
import os
import numpy as np

for _p in ("/opt/trn_rl_repo", "/root/.axon_site/_ro/trn_rl_repo"):
    if os.path.isdir(_p) and _p not in sys.path:
        sys.path.insert(0, _p)

import ml_dtypes  # noqa: E402

# ----------------------------------------------------------------------------
# problem constants (hardcoded per spec)
C = 64
H = W = 112
N_FULL = 32
NCORES = 8
N_PER_CORE = N_FULL // NCORES        # 4 images
NPAIRS = N_PER_CORE // 2             # 2 image pairs (2 images share 128 parts)
WP = 114                             # padded row width
LEAD = 1                             # one leading zero element
TLEN = LEAD + WP * WP + 2            # padded image + tail slack = 12999
HWPIX = H * W                        # 12544
ROWS_PER_CHUNK = 4
NCHUNKS = H // ROWS_PER_CHUNK        # 28
CHUNK_N = ROWS_PER_CHUNK * W         # 448 psum elements (pad cols excluded)
PIECE_ROWS = [8, 8, 16, 28, 28, 24]  # x load/convert piece sizes (rows)
PIECE_START = [0, 8, 16, 32, 60, 88]
# piece p is prefetched while chunk PIECE_EMIT[p] runs
PIECE_EMIT = {1: 3, 7: 4, 14: 5}
CHUNK_N2 = ROWS_PER_CHUNK * WP       # 456: conv2 psum incl pad cols (dr mode)
MAGIC = 12582912.0                   # 1.5 * 2**23, forces RNE round-to-int
EPS = 1e-5

_CACHE = {}


# ----------------------------------------------------------------------------
# host-side preprocessing
def _quantize_weight_int(w):
    """DoReFa 4-bit weight quantization, returning 15*q(w) (odd ints in
    [-15,15]) as float32.  Computed with jax on CPU to match the
    reference bit-for-bit."""
    import jax
    import jax.numpy as jnp

    cpu = jax.devices("cpu")[0]
    with jax.default_device(cpu):
        t = jnp.tanh(jnp.asarray(np.asarray(w, dtype=np.float32)))
        t = t / (2.0 * jnp.max(jnp.abs(t))) + 0.5
        m = jnp.round(t * 15.0)
        out = 2.0 * m - 15.0
        return np.asarray(out, dtype=np.float32)


def _block_diag_taps(w_int):
    """w_int: (Cout, Cin, 3, 3) float int values.  Returns (128, 9*128)
    bf16: for tap t, lhsT[k, t*128+m] with block-diagonal two-image
    structure; lhsT[cin, cout] = w[cout, cin] in each 64x64 block."""
    bd = np.zeros((128, 9, 128), dtype=np.float32)
    for t in range(9):
        ky, kx = divmod(t, 3)
        blk = w_int[:, :, ky, kx].T  # (Cin, Cout)
        bd[0:64, t, 0:64] = blk
        bd[64:128, t, 64:128] = blk
    return bd.reshape(128, 9 * 128).astype(ml_dtypes.bfloat16)


def _dr_weights(w_int):
    """fp8 DoubleRow conv2 weights: per kx, taps (ky=0, ky=1) interleaved
    [k, 2, m]; plus plain [k, m] blocks for ky=2."""
    f8 = ml_dtypes.float8_e4m3fn
    dr = np.zeros((128, 3, 2, 128), dtype=np.float32)
    ky2 = np.zeros((128, 3, 128), dtype=np.float32)
    for kx in range(3):
        for i in range(2):
            blk = w_int[:, :, i, kx].T
            dr[0:64, kx, i, 0:64] = blk
            dr[64:128, kx, i, 64:128] = blk
        blk2 = w_int[:, :, 2, kx].T
        ky2[0:64, kx, 0:64] = blk2
        ky2[64:128, kx, 64:128] = blk2
    return dr.reshape(128, 768).astype(f8), ky2.reshape(128, 384).astype(f8)


def _drc_weights(w_int):
    """conv2 column-pair DR weights: lanes = taps (ky=2, kx=0) and
    (ky=2, kx=1); the ifmap lanes are the same a1 row shifted one col."""
    f8 = ml_dtypes.float8_e4m3fn
    c = np.zeros((128, 2, 128), dtype=np.float32)
    for i in range(2):
        blk = w_int[:, :, 2, i].T
        c[0:64, i, 0:64] = blk
        c[64:128, i, 64:128] = blk
    return c.reshape(128, 256).astype(f8)


def _dr1_weights(w_int):
    """conv1 DR weights: per tap, both lanes carry the same block-diag
    [k, 2, m] (lanes multiply the h1/h2 fp8 parts of x)."""
    f8 = ml_dtypes.float8_e4m3fn
    dr = np.zeros((128, 9, 2, 128), dtype=np.float32)
    for t in range(9):
        ky, kx = divmod(t, 3)
        blk = w_int[:, :, ky, kx].T
        for i in range(2):
            dr[0:64, t, i, 0:64] = blk
            dr[64:128, t, i, 64:128] = blk
    return dr.reshape(128, 9 * 256).astype(f8)


def _bn_fold(gamma, beta, mean, var):
    gamma = np.asarray(gamma, np.float32)
    beta = np.asarray(beta, np.float32)
    mean = np.asarray(mean, np.float32)
    var = np.asarray(var, np.float32)
    inv = gamma / np.sqrt(var + np.float32(EPS))
    b = beta - mean * inv
    return inv.astype(np.float32), b.astype(np.float32)


def _tile2(v):
    """(64,) -> (128,1) stacked for the two images on the partitions."""
    return np.concatenate([v, v]).reshape(128, 1).astype(np.float32)


# ----------------------------------------------------------------------------
# walrus workaround: CTRL instructions (Drain) accept only ONE sync wait
def _install_tile_patch(tile_mod):
    import bass_rust

    if getattr(tile_mod.TileContext, "_drain_split_patch", False):
        return

    def _patched(self, tick_clock, wait_clock):
        nc = self.nc
        drain_inst = nc.sync.drain()
        wait_clock.add_sem_waits(
            drain_inst.ins, bass_rust.ScopedClock({None: tick_clock.global_clock})
        )
        si = drain_inst.ins.sync_info
        waits = list(si.on_wait or []) if si is not None else []
        if len(waits) > 1:
            si.on_wait = waits[:1]
            for w in waits[1:]:
                d2 = nc.sync.drain()
                d2.ins.sync_info = bass_rust.SyncInfo(on_wait=[w], on_update=[])
        nc.all_engine_barrier()
        assert self.sems is not None
        popped = nc._tile_sem_poison_stack.pop()
        assert popped is self._sem_poison
        nc.clear_and_free_semaphores(list(self.sems.allocated().values()))
        nc.all_engine_barrier()

    tile_mod.TileContext._drain_and_barrier = _patched
    tile_mod.TileContext._drain_split_patch = True


def _split_multiwaits(nc, max_waits=1):
    """This walrus build allows only ONE sync wait per instruction
    (setupSyncWait raises 'Too many sync wait commands' at 2 for both
    CTRL and MM structs).  Move excess waits onto injected same-engine
    NoOps immediately before the over-limit instruction — semantically
    identical since the engine executes its stream in order."""
    import bass_rust
    from concourse import mybir

    n_split = 0
    for fn in nc.m.functions:
        for bb in fn.blocks:
            insts = bb.instructions  # live list
            new = None
            for idx, inst in enumerate(insts):
                si = inst.sync_info
                waits = list(si.on_wait) if (si is not None and si.on_wait) else []
                if len(waits) <= max_waits:
                    if new is not None:
                        new.append(inst)
                    continue
                if new is None:
                    new = list(insts[:idx])
                excess, keep = waits[:-max_waits], waits[-max_waits:]
                for w in excess:
                    nop = mybir.InstNoOp(
                        name=nc.get_next_instruction_name(), ins=[], outs=[]
                    )
                    nop.engine = inst.engine
                    nop.sync_info = bass_rust.SyncInfo(on_wait=[w], on_update=[])
                    nc.register_instruction(nop, overwrite=True)
                    new.append(nop)
                si.on_wait = keep
                new.append(inst)
                n_split += 1
            if new is not None:
                bb.instructions = new
    return n_split


# ----------------------------------------------------------------------------
# device program
DEFAULT_MODE = os.environ.get("BASS_CONV1_MODE", "f16")


def build_program(repeat=1, mode=None):
    mode = mode or DEFAULT_MODE
    key = ("nc", repeat, mode)
    if key in _CACHE:
        return _CACHE[key]

    import concourse.bass as bass
    import concourse.tile as tile
    from concourse import mybir

    _install_tile_patch(tile)

    f32 = mybir.dt.float32
    bf16 = mybir.dt.bfloat16
    Alu = mybir.AluOpType
    Act = mybir.ActivationFunctionType

    nc = bass.Bass("TRN2", debug=False, enable_asserts=False)

    f32r = mybir.dt.float32r
    fp8 = mybir.dt.float8e4
    f16 = mybir.dt.float16
    dr = (mode in ("split_dr", "dr_mid", "f16"))
    drmid = (mode == "dr_mid")
    w1dt = f32r if mode == "fp32r" else bf16
    xdt = {"fp32r": f32r, "f16": f16}.get(mode, f32)
    xd = nc.dram_tensor("x", [NPAIRS, 128, HWPIX], xdt, kind="ExternalInput").ap()
    if drmid:
        w1drd = nc.dram_tensor("w1dr", [128, 9 * 256], fp8,
                               kind="ExternalInput").ap()
    w1d = nc.dram_tensor("w1bd", [128, 9 * 128], w1dt, kind="ExternalInput").ap()
    if dr:
        w2d = nc.dram_tensor("w2dr", [128, 768], fp8, kind="ExternalInput").ap()
        w2k2d = nc.dram_tensor("w2ky2", [128, 384], fp8, kind="ExternalInput").ap()
        w2cd = nc.dram_tensor("w2c", [128, 256], fp8, kind="ExternalInput").ap()
    else:
        w2d = nc.dram_tensor("w2bd", [128, 9 * 128], bf16, kind="ExternalInput").ap()
    s1d = nc.dram_tensor("s1", [128, 1], f32, kind="ExternalInput").ap()
    b1d = nc.dram_tensor("b1c", [128, 1], f32, kind="ExternalInput").ap()
    s2d = nc.dram_tensor("s2", [128, 1], f32, kind="ExternalInput").ap()
    b2d = nc.dram_tensor("b2c", [128, 1], f32, kind="ExternalInput").ap()
    odt = fp8 if mode == "f16" else f32  # f16 mode: 15*out ints, host /15
    od = nc.dram_tensor("out", [NPAIRS, 128, HWPIX], odt, kind="ExternalOutput").ap()

    def img_rows(tile_ap, r0, r1, c0=1, c1=113):
        """strided view of padded-image tile rows [r0,r1) cols [c0,c1)"""
        flat = tile_ap[:, LEAD + r0 * WP: LEAD + r1 * WP]
        return flat.rearrange("p (r c) -> p r c", c=WP)[:, :, c0:c1]

    def img_rows_at(tile_ap, base, r0, r1, c0=1, c1=113):
        flat = tile_ap[:, base + LEAD + r0 * WP: base + LEAD + r1 * WP]
        return flat.rearrange("p (r c) -> p r c", c=WP)[:, :, c0:c1]

    def tap_rhs(tile_ap, y0, ky, kx):
        """[128, 4, 112] matmul rhs for tap (ky,kx) of out rows y0..y0+3"""
        off = LEAD + (y0 + ky) * WP + kx
        flat = tile_ap[:, off: off + ROWS_PER_CHUNK * WP]
        return flat.rearrange("p (r c) -> p r c", c=WP)[:, :, 0:112]

    with tile.TileContext(nc) as tc:
        import contextlib

        with contextlib.ExitStack() as ctx:
            consts = ctx.enter_context(tc.tile_pool(name="consts", bufs=1))
            bigp = ctx.enter_context(tc.tile_pool(name="big", bufs=1))
            xpp = ctx.enter_context(tc.tile_pool(name="xp", bufs=2))
            workp = ctx.enter_context(tc.tile_pool(name="work", bufs=2))
            outp = ctx.enter_context(tc.tile_pool(name="outs", bufs=4))
            ps1p = ctx.enter_context(tc.tile_pool(name="ps1", bufs=4, space="PSUM"))
            ps2p = ctx.enter_context(tc.tile_pool(name="ps2", bufs=4, space="PSUM"))

            w1t = consts.tile([128, 9 * 128], w1dt, tag="w1t")
            nc.sync.dma_start(w1t[:], w1d[:])
            if drmid:
                w1drt = consts.tile([128, 9 * 256], fp8, tag="w1drt")
                nc.sync.dma_start(w1drt[:], w1drd[:])
            if dr:
                w2t = consts.tile([128, 768], fp8, tag="w2t")
                nc.sync.dma_start(w2t[:], w2d[:])
                w2k2t = consts.tile([128, 384], fp8, tag="w2k2t")
                nc.sync.dma_start(w2k2t[:], w2k2d[:])
                w2ct = consts.tile([128, 256], fp8, tag="w2ct")
                nc.sync.dma_start(w2ct[:], w2cd[:])
            else:
                w2t = consts.tile([128, 9 * 128], bf16, tag="w2t")
                nc.sync.dma_start(w2t[:], w2d[:])
            s1t = consts.tile([128, 1], f32, tag="s1t")
            nc.sync.dma_start(s1t[:], s1d[:])
            b1t = consts.tile([128, 1], f32, tag="b1t")
            nc.sync.dma_start(b1t[:], b1d[:])
            s2t = consts.tile([128, 1], f32, tag="s2t")
            nc.sync.dma_start(s2t[:], s2d[:])
            b2t = consts.tile([128, 1], f32, tag="b2t")
            nc.sync.dma_start(b2t[:], b2d[:])

            for pair_rep in range(NPAIRS * repeat):
                pair = pair_rep % NPAIRS
                if drmid:
                    h12 = bigp.tile([128, 2 * TLEN], fp8, tag="h12")
                    lo2 = bigp.tile([128, TLEN], bf16, tag="lo2")
                    pad_tiles = (lo2,)
                elif mode == "f16":
                    xf = bigp.tile([128, TLEN], f16, tag="xf", bufs=2)
                    pad_tiles = (xf,)
                elif mode != "fp32r":
                    # double-buffered so pair p+1's x conversion overlaps
                    # pair p's tail instead of WAR-stalling behind it
                    xhi = bigp.tile([128, TLEN], bf16, tag="xhi", bufs=2)
                    xlo = bigp.tile([128, TLEN], bf16, tag="xlo", bufs=2)
                    pad_tiles = (xhi, xlo)
                else:
                    xt = bigp.tile([128, TLEN], f32r, tag="xt")
                    pad_tiles = (xt,)
                if dr:
                    a1 = bigp.tile([128, TLEN], fp8, tag="a1")
                else:
                    a1 = bigp.tile([128, TLEN], bf16, tag="a1")

                # zero pad regions (lead + row0, row113 + tail, col 0, col 113)
                def _ms(ap):
                    if ap.dtype == f32r:
                        ap = ap.bitcast(f32)
                    nc.gpsimd.memset(ap, 0.0)

                def _zero_pads(t_, base):
                    _ms(t_[:, base: base + LEAD + WP])
                    _ms(t_[:, base + LEAD + 113 * WP: base + TLEN])
                    colv = t_[:, base + LEAD: base + LEAD + WP * WP].rearrange(
                        "p (r c) -> p r c", c=WP
                    )
                    _ms(colv[:, :, 0:1])
                    _ms(colv[:, :, 113:114])

                for t_ in pad_tiles:
                    _zero_pads(t_, 0)
                if drmid:
                    _zero_pads(h12, 0)
                    _zero_pads(h12, TLEN)
                _zero_pads(a1, 0)

                def convert_piece(p):
                    pr, p0 = PIECE_ROWS[p], PIECE_START[p]
                    src_flat = xd[pair][:, p0 * W: (p0 + pr) * W]
                    src_ap = src_flat.rearrange("p (r c) -> p r c", c=W)
                    r0 = p0 + 1  # tile row of first image row
                    if mode == "f16":
                        nc.sync.dma_start(img_rows(xf, r0, r0 + pr), src_ap)
                    elif drmid:
                        xp_ = xpp.tile([128, 28 * W], f32, tag="xp")
                        nc.sync.dma_start(xp_[:, 0: pr * W], src_flat)
                        xpv = xp_[:, 0: pr * W].rearrange("p (r c) -> p r c", c=W)
                        h1v = img_rows(h12, r0, r0 + pr)
                        nc.gpsimd.dma_start(h1v, src_ap)  # cast f32->fp8
                        ra = xpp.tile([128, 28 * W], f32, tag="ra")
                        rav = ra[:, 0: pr * W].rearrange("p (r c) -> p r c", c=W)
                        nc.gpsimd.tensor_sub(rav, xpv, h1v)
                        h2v = img_rows_at(h12, TLEN, r0, r0 + pr)
                        nc.scalar.activation(h2v, rav, Act.Copy, bias=0.0, scale=1.0)
                        lo2v = img_rows(lo2, r0, r0 + pr)
                        nc.vector.tensor_sub(lo2v, rav, h2v)
                    elif mode != "fp32r":
                        xp_ = xpp.tile([128, 28 * W], f32, tag="xp")
                        nc.sync.dma_start(xp_[:, 0: pr * W], src_flat)
                        xpv = xp_[:, 0: pr * W].rearrange("p (r c) -> p r c", c=W)
                        hiv = img_rows(xhi, r0, r0 + pr)
                        lov = img_rows(xlo, r0, r0 + pr)
                        # casting DMA (fp32->bf16) produces x_hi; x_lo picks
                        # up whatever rounding the cast used.
                        nc.gpsimd.dma_start(hiv, src_ap)
                        nc.vector.tensor_sub(lov, xpv, hiv)
                    else:
                        nc.sync.dma_start(img_rows(xt, r0, r0 + pr), src_ap)

                def conv1_chunk(k):
                    y0 = k * ROWS_PER_CHUNK
                    CN1 = CHUNK_N2 if drmid else CHUNK_N
                    ps = ps1p.tile([128, CN1], f32, tag="ps1")
                    if drmid:
                        h12two = h12[:].rearrange("p (two L) -> p two L", two=2)
                        for t in range(9):
                            ky, kx = divmod(t, 3)
                            off = (y0 + ky) * WP + kx
                            nc.tensor.matmul(
                                ps[:],
                                w1drt[:, t * 256: (t + 1) * 256].rearrange(
                                    "p (two m) -> p two m", two=2),
                                h12two[:, :, off: off + CHUNK_N2],
                                start=(t == 0),
                                stop=False,
                                perf_mode=mybir.MatmulPerfMode.DoubleRow,
                            )
                        for t in range(9):
                            ky, kx = divmod(t, 3)
                            off = (y0 + ky) * WP + kx
                            nc.tensor.matmul(
                                ps[:],
                                w1t[:, t * 128: (t + 1) * 128],
                                lo2[:, off: off + CHUNK_N2],
                                start=False,
                                stop=(t == 8),
                            )
                    else:
                        srcs = ((xf,) if mode == "f16" else
                                (xt,) if mode == "fp32r" else (xhi, xlo))
                        n_mm = 9 * len(srcs)
                        i = 0
                        for t in range(9):
                            ky, kx = divmod(t, 3)
                            for s_ in srcs:
                                nc.tensor.matmul(
                                    ps[:],
                                    w1t[:, t * 128: (t + 1) * 128],
                                    tap_rhs(s_, y0, ky, kx),
                                    start=(i == 0),
                                    stop=(i == n_mm - 1),
                                )
                                i += 1
                    # t1 = psum * inv1 + 15*b1; then +MAGIC rounds to
                    # nearest-even integer, clip in the shifted domain,
                    # subtract MAGIC (round/clip commute here).
                    t1 = workp.tile([128, CN1], f32, tag="t1")
                    nc.scalar.activation(
                        t1[:], ps[:], Act.Identity, bias=b1t[:], scale=s1t[:]
                    )
                    m1 = workp.tile([128, CN1], f32, tag="m1")
                    nc.vector.tensor_scalar(
                        m1[:], t1[:], MAGIC, MAGIC, Alu.add, Alu.max
                    )
                    a1v = img_rows(a1, y0 + 1, y0 + 5)
                    if drmid:
                        m1v = m1[:].rearrange("p (r c) -> p r c", c=WP)[:, :, 1:113]
                    else:
                        m1v = m1[:].rearrange("p (r c) -> p r c", c=W)
                    nc.vector.tensor_scalar(
                        a1v, m1v, MAGIC + 15.0, -MAGIC, Alu.min, Alu.add
                    )


                def conv2_chunk(k):
                    y0 = k * ROWS_PER_CHUNK
                    if dr:
                        import bass_rust as _br
                        ps = ps2p.tile([128, CHUNK_N2], f32, tag="ps2")
                        for kx in range(3):
                            off = y0 * WP + kx
                            rrhs = a1[:, off: off + WP + CHUNK_N2].copy()
                            rrhs.ap = _br.VecI64Pair(
                                [[TLEN, 128], [WP, 2], [1, CHUNK_N2]])
                            nc.tensor.matmul(
                                ps[:],
                                w2t[:, kx * 256: (kx + 1) * 256].rearrange(
                                    "p (two m) -> p two m", two=2),
                                rrhs,
                                start=(kx == 0),
                                stop=False,
                                perf_mode=mybir.MatmulPerfMode.DoubleRow,
                            )
                        # (ky=2, kx=0|1) as one DR matmul with overlapping
                        # lanes: lane i reads a1 at a one-element offset
                        off = (y0 + 2) * WP
                        crhs = a1[:, off: off + CHUNK_N2 + 1].copy()
                        crhs.ap = _br.VecI64Pair(
                            [[TLEN, 128], [1, 2], [1, CHUNK_N2]])
                        nc.tensor.matmul(
                            ps[:],
                            w2ct[:].rearrange("p (two m) -> p two m", two=2),
                            crhs,
                            start=False, stop=False,
                            perf_mode=mybir.MatmulPerfMode.DoubleRow,
                        )
                        off2 = (y0 + 2) * WP + 2
                        nc.tensor.matmul(
                            ps[:],
                            w2k2t[:, 2 * 128: 3 * 128],
                            a1[:, off2: off2 + CHUNK_N2],
                            start=False,
                            stop=True,
                        )
                    else:
                        ps = ps2p.tile([128, CHUNK_N], f32, tag="ps2")
                        for t in range(9):
                            ky, kx = divmod(t, 3)
                            nc.tensor.matmul(
                                ps[:],
                                w2t[:, t * 128: (t + 1) * 128],
                                tap_rhs(a1, y0, ky, kx),
                                start=(t == 0),
                                stop=(t == 8),
                            )
                    NC2 = CHUNK_N2 if dr else CHUNK_N
                    # t2 = psum * (inv2/15) + 15*b2
                    t2 = workp.tile([128, NC2], f32, tag="t2")
                    nc.scalar.activation(
                        t2[:], ps[:], Act.Identity, bias=b2t[:], scale=s2t[:]
                    )
                    # + 15*x  (x = x_hi + x_lo)
                    if drmid:
                        xbase = (y0 + 1) * WP + 1  # center tap, incl pad cols
                        v1 = workp.tile([128, NC2], f32, tag="v1")
                        nc.vector.scalar_tensor_tensor(
                            v1[:], h12[:, xbase: xbase + CHUNK_N2], 15.0, t2[:],
                            Alu.mult, Alu.add)
                        v0 = workp.tile([128, NC2], f32, tag="v0")
                        nc.vector.scalar_tensor_tensor(
                            v0[:], h12[:, TLEN + xbase: TLEN + xbase + CHUNK_N2],
                            15.0, v1[:], Alu.mult, Alu.add)
                        v2 = workp.tile([128, NC2], f32, tag="v2")
                        nc.vector.scalar_tensor_tensor(
                            v2[:], lo2[:, xbase: xbase + CHUNK_N2], 15.0, v0[:],
                            Alu.mult, Alu.add)
                    elif mode == "f16":
                        xbase = (y0 + 1) * WP + 1  # center tap, incl pad cols
                        v2 = workp.tile([128, NC2], f32, tag="v2")
                        nc.vector.scalar_tensor_tensor(
                            v2[:], xf[:, xbase: xbase + CHUNK_N2], 15.0, t2[:],
                            Alu.mult, Alu.add)
                    elif dr:
                        xbase = (y0 + 1) * WP + 1  # center tap, incl pad cols
                        xh = xhi[:, xbase: xbase + CHUNK_N2]
                        xl = xlo[:, xbase: xbase + CHUNK_N2]
                        v1 = workp.tile([128, NC2], f32, tag="v1")
                        nc.vector.scalar_tensor_tensor(
                            v1[:], xh, 15.0, t2[:], Alu.mult, Alu.add)
                        v2 = workp.tile([128, NC2], f32, tag="v2")
                        nc.vector.scalar_tensor_tensor(
                            v2[:], xl, 15.0, v1[:], Alu.mult, Alu.add)
                    elif mode == "split":
                        t2v = t2[:].rearrange("p (r c) -> p r c", c=W)
                        v1 = workp.tile([128, NC2], f32, tag="v1")
                        nc.vector.scalar_tensor_tensor(
                            v1[:].rearrange("p (r c) -> p r c", c=W),
                            tap_rhs(xhi, y0, 1, 1), 15.0, t2v,
                            Alu.mult, Alu.add,
                        )
                        v2 = workp.tile([128, NC2], f32, tag="v2")
                        nc.vector.scalar_tensor_tensor(
                            v2[:].rearrange("p (r c) -> p r c", c=W),
                            tap_rhs(xlo, y0, 1, 1), 15.0,
                            v1[:].rearrange("p (r c) -> p r c", c=W),
                            Alu.mult, Alu.add,
                        )
                    else:
                        t2v = t2[:].rearrange("p (r c) -> p r c", c=W)
                        v2 = workp.tile([128, NC2], f32, tag="v2")
                        nc.vector.scalar_tensor_tensor(
                            v2[:].rearrange("p (r c) -> p r c", c=W),
                            tap_rhs(xt, y0, 1, 1).bitcast(f32), 15.0, t2v,
                            Alu.mult, Alu.add,
                        )
                    u2 = workp.tile([128, NC2], f32, tag="u2")
                    if mode == "f16":
                        # round+clip in the MAGIC-shifted domain, two 2-ALU
                        # DVE ops; fp8 output of the integer 15*out (host
                        # divides by 15).
                        nc.vector.tensor_scalar(
                            u2[:], v2[:], MAGIC, MAGIC, Alu.add, Alu.max)
                        o = outp.tile([128, NC2], fp8, tag="o")
                        nc.vector.tensor_scalar(
                            o[:], u2[:], MAGIC + 15.0, -MAGIC, Alu.min, Alu.add)
                    else:
                        nc.vector.tensor_scalar(
                            u2[:], v2[:], 0.0, 15.0, Alu.max, Alu.min)
                        m2 = workp.tile([128, NC2], f32, tag="m2")
                        nc.scalar.activation(
                            m2[:], u2[:], Act.Copy, bias=MAGIC, scale=1.0)
                        o = outp.tile([128, NC2], f32, tag="o")
                        nc.vector.tensor_scalar(
                            o[:], m2[:], -MAGIC, 1.0 / 15.0, Alu.add, Alu.mult
                        )
                    if dr:
                        ov = o[:].rearrange("p (r c) -> p r c", c=WP)[:, :, 1:113]
                        dst = od[pair].rearrange(
                            "p (r c) -> p r c", c=W)[:, y0: y0 + 4, :]
                        nc.sync.dma_start(dst, ov)
                    else:
                        nc.sync.dma_start(
                            od[pair][:, y0 * W: y0 * W + CHUNK_N], o[:]
                        )

                convert_piece(0)
                convert_piece(1)
                convert_piece(2)
                conv1_chunk(0)
                for k in range(1, NCHUNKS):
                    if k in PIECE_EMIT:
                        convert_piece(PIECE_EMIT[k])
                    conv1_chunk(k)
                    conv2_chunk(k - 1)
                conv2_chunk(NCHUNKS - 1)

    _split_multiwaits(nc)
    _CACHE[key] = nc
    return nc


# ----------------------------------------------------------------------------
def _make_in_maps(x, w1, gamma1, beta1, mean1, var1, w2, gamma2, beta2,
                  mean2, var2, mode=None):
    mode = mode or DEFAULT_MODE
    x = np.ascontiguousarray(np.asarray(x, dtype=np.float32))
    w1i = _quantize_weight_int(np.asarray(w1))
    w2i = _quantize_weight_int(np.asarray(w2))
    w1bd = _block_diag_taps(w1i)
    if mode == "fp32r":
        w1bd = np.asarray(w1bd, dtype=np.float32)
    w2bd = _block_diag_taps(w2i)
    inv1, b1 = _bn_fold(gamma1, beta1, mean1, var1)
    inv2, b2 = _bn_fold(gamma2, beta2, mean2, var2)
    s1 = _tile2(inv1)                      # psum1 * inv1 + 15*b1
    b1c = _tile2(np.float32(15.0) * b1)
    s2 = _tile2(inv2 / np.float32(15.0))   # psum2 * inv2/15 + 15*b2
    b2c = _tile2(np.float32(15.0) * b2)

    if mode in ("split_dr", "dr_mid", "f16"):
        w2dr, w2ky2 = _dr_weights(w2i)
    if mode == "f16":
        x = x.astype(np.float16)
    in_maps = []
    for core in range(NCORES):
        xs = x[core * N_PER_CORE: (core + 1) * N_PER_CORE]
        xs = xs.reshape(NPAIRS, 128, HWPIX)
        m = {"x": xs, "w1bd": w1bd,
             "s1": s1, "b1c": b1c, "s2": s2, "b2c": b2c}
        if mode in ("split_dr", "dr_mid", "f16"):
            m["w2dr"] = w2dr
            m["w2ky2"] = w2ky2
            m["w2c"] = _drc_weights(w2i)
        else:
            m["w2bd"] = w2bd
        if mode == "dr_mid":
            m["w1dr"] = _dr1_weights(w1i)
        in_maps.append(m)
    return in_maps


def _install_hook_logging():
    """Surface neuronx-cc hook exceptions (PJRT swallows the traceback)."""
    if _CACHE.get("hooked"):
        return
    import traceback
    from concourse import bass2jax

    bass2jax.install_neuronx_cc_hook()
    try:
        import libneuronxla
    except ImportError:
        return
    orig = libneuronxla.neuronx_cc

    def wrapped(*a, **k):
        try:
            return orig(*a, **k)
        except BaseException:
            traceback.print_exc()
            raise

    libneuronxla.neuronx_cc = wrapped
    bass2jax.install_neuronx_cc_hook = lambda: None
    _CACHE["hooked"] = True


def run(in_maps, trace=False, mode=None):
    from concourse import bass_utils

    _install_hook_logging()
    nc = build_program(mode=mode)
    res = bass_utils.run_bass_kernel_spmd(
        nc, in_maps, core_ids=list(range(NCORES)), trace=trace
    )
    return res


def unshard(res, mode=None):
    mode = mode or DEFAULT_MODE
    outs = []
    for core in range(NCORES):
        o = np.asarray(res.results[core]["out"], dtype=np.float32)
        if mode == "f16":  # device wrote integer 15*out as fp8
            o *= np.float32(1.0 / 15.0)
        outs.append(o.reshape(N_PER_CORE, C, H, W))
    return np.concatenate(outs, axis=0)


def kernel(x, w1, gamma1, beta1, mean1, var1, w2, gamma2, beta2, mean2, var2):
    in_maps = _make_in_maps(x, w1, gamma1, beta1, mean1, var1,
                            w2, gamma2, beta2, mean2, var2)
    res = run(in_maps, trace=False)  # DEFAULT_MODE for both maps and program
    return unshard(res)

